# revision 25
# baseline (speedup 1.0000x reference)
"""Causal single-head attention on 8 trn2 NeuronCores.

B=4, S=2048, D_MODEL=1024, D_HEAD=64, fp32 in/out.

Sharding: 2 cores per batch. Core half h=0 owns query tiles {0..3,12..15}
(rows 0:512, 1536:2048), h=1 owns {4..11} (rows 512:1536); both own 68
causal 128x128 blocks. The host feeds each core its batch's embeddings
already TRANSPOSED to E^T [dm, s] in bf16 with columns permuted so own
query rows come first - no on-device transposes/casts of E at all.

Per-core pipeline (identical SPMD program):
  Projections from E^T with packed weights: own chunks use [Wq/8|Wk]
  (M=128, full PE array) plus V; other chunks use [Wk|Wv]. Outputs land
  in a stacked QKT sbuf [128, S] (rows 0:64 Q^T, 64:128 K^T). V tiles are
  PE-transposed into Vp [128k, 16, 65] with a ones column (denominator).
  Attention over local key tiles kt:
    kt 0..3  : scores vs both slots (N=1024 via 2 matmuls into one 2-bank
               PSUM), one exp, tri-mask multiply on slot0 cols
    kt 4..7  : slot1 only (N=512), tri mask
    kt 8..11 : both slots; slot0 multiplied by per-core 0/1 gate vector
    kt 12..15: slot1 only; per-core 0/-30000 exp bias kills it on h=1
  PV accumulates out^T [65, 512] per slot in PSUM (col 64 = sum exp);
  host does the final divide + transpose + scatter.
"""

import sys

if "/opt/trn_rl_repo" not in sys.path:
    sys.path.insert(0, "/opt/trn_rl_repo")

import numpy as np

B, S, D, H = 4, 2048, 1024, 64
P = 128
KO = D // P          # 8 dmodel chunks
NT = S // P          # 16 seq tiles
NEG = -30000.0


def _halves():
    return [[(0, 512), (1536, 2048)], [(512, 1536)]]


def _build_program():
    import concourse.bacc as bacc
    import concourse.mybir as mybir
    import concourse.tile as tile

    f32 = mybir.dt.float32
    bf16 = mybir.dt.bfloat16
    AF = mybir.ActivationFunctionType
    ALU = mybir.AluOpType

    nc = bacc.Bacc()
    # et layout [chunk, partition, KO*512]: 8 KB contiguous per partition
    # per chunk -> big DMA descriptors (1 KB descriptors run ~21 GB/s/queue)
    et = nc.declare_dram_parameter("et", [4, P, KO * 512], bf16, isOutput=False)
    # chunk 0 split into four 128-col sub-chunks for a fast pipeline start
    et0 = nc.declare_dram_parameter("et0", [4, P, KO, P], bf16, isOutput=False)
    # [Wv|Wk] in cols 0:128, Wq/8 in cols 128:192
    wts = nc.declare_dram_parameter("wts", [P, KO, 192], bf16, isOutput=False)
    # cols: bq/8 | bk | g8 | g12n | bv (bv only rows 0:64 meaningful)
    bias4 = nc.declare_dram_parameter("bias4", [P, 5], f32, isOutput=False)
    # cols 0:2048 = tri masks (4 x 512), cols 2048:2112 = identity (rows 0:64)
    mi = nc.declare_dram_parameter("mi", [P, 4 * 512 + H], bf16, isOutput=False)
    out = nc.declare_dram_parameter("out", [H + 1, 1024], f32, isOutput=True)

    from contextlib import ExitStack

    with tile.TileContext(nc) as tc, ExitStack() as ctx:
        cpool = ctx.enter_context(tc.tile_pool(name="const", bufs=1))
        vtp = ctx.enter_context(tc.tile_pool(name="vt", bufs=2))
        ptp = ctx.enter_context(tc.tile_pool(name="pt", bufs=8))
        psb = ctx.enter_context(tc.tile_pool(name="psb", bufs=2, space="PSUM"))

        # --- input DMAs. Each dma_start spreads across all 16 queues at
        # ~300 GB/s aggregate but pays ~0.5us fixed cost, and queues come
        # up staggered over the first ~7us - so: few dma_starts, the
        # startup-critical ones (weights, chunk0 sub-chunks) first.
        wts_sb = cpool.tile([P, KO, 192], bf16, tag="wts")
        nc.sync.dma_start(wts_sb[:], wts[:])
        ET0 = cpool.tile([P, 4, KO, P], bf16, tag="ET0")
        for j in range(4):
            nc.sync.dma_start(ET0[:, j], et0[j])
        # [partition, chunk, ko, 512]; chunk 0 lives in ET0 instead
        ET = cpool.tile([P, 4, KO, 512], bf16, tag="ET")
        nc.sync.dma_start(ET[:, 1, :, :], et[1, :, :])
        bias_sb = cpool.tile([P, 5], f32, tag="bias4")
        nc.sync.dma_start(bias_sb[:], bias4[:])
        mi_sb = cpool.tile([P, 4 * 512 + H], bf16, tag="mi")
        nc.sync.dma_start(mi_sb[:], mi[:])
        nc.sync.dma_start(ET[:, 2, :, :], et[2, :, :])
        nc.sync.dma_start(ET[:, 3, :, :], et[3, :, :])
        bq_sb = bias_sb[:, 0:1]
        bk_sb = bias_sb[:, 1:2]
        g8_sb = bias_sb[:, 2:3]
        g12_sb = bias_sb[:, 3:4]
        bv_sb = bias_sb[:H, 4:5]
        id_sb = mi_sb[:H, 4 * 512:4 * 512 + H]

        def mask_ap(j):
            return mi_sb[:, j * 512:(j + 1) * 512]

        # Q^T and K^T both live on partitions 64:128 (matmul requires lhsT
        # and rhs to share a base partition; the packed [Wv|Wk] projection
        # puts K^T on PSUM rows 64:128 and DVE copies cannot shift rows).
        QT = cpool.tile([P, 1024], bf16, tag="QT")
        KT = cpool.tile([P, S], bf16, tag="KT")
        Vp = cpool.tile([P, NT, H + 1], bf16, tag="Vp")
        nc.vector.memset(Vp[:, :, H:H + 1], 1.0)
        o_sb = cpool.tile([P, 1024], f32, tag="osb")

        def vtranspose(vt, cc):
            for t in range(4):
                kt = cc * 4 + t
                pvt = psb.tile([P, H], bf16, tag="pj", name=f"pvt_{kt}")
                nc.tensor.transpose(
                    pvt[:], vt[:, t * P:(t + 1) * P], id_sb[:]
                )
                nc.vector.tensor_copy(Vp[:, kt, :H], pvt[:])

        vts = [None] * 4

        def vk_chunk0():
            # chunk 0 as four 128-col accumulation groups so the first
            # matmul only waits for the first 256 KB sub-chunk
            ps = psb.tile([P, 512], f32, tag="pj", name="vk_ps_0")
            for j in range(4):
                for ko in range(KO):
                    nc.tensor.matmul(
                        ps[:, j * P:(j + 1) * P], wts_sb[:, ko, 0:128],
                        ET0[:, j, ko, :],
                        start=(ko == 0), stop=(ko == KO - 1),
                        skip_group_check=True,
                    )
            nc.vector.tensor_scalar_add(
                KT[H:P, 0:512], ps[H:P, :], bk_sb[H:P]
            )
            vt = vtp.tile([H, 512], bf16, tag="vt", name="vt_0")
            nc.vector.tensor_scalar_add(vt[:], ps[:H, :], bv_sb[:])
            vts[0] = vt

        def q_chunk0():
            ps = psb.tile([P, 512], f32, tag="pj", name="q_ps_0")
            for j in range(4):
                for ko in range(KO):
                    nc.tensor.matmul(
                        ps[H:P, j * P:(j + 1) * P], wts_sb[:, ko, 128:192],
                        ET0[:, j, ko, :],
                        start=(ko == 0), stop=(ko == KO - 1),
                        skip_group_check=True,
                    )
            nc.vector.tensor_scalar_add(
                QT[H:P, 0:512], ps[H:P, :], bq_sb[H:P]
            )

        def vk_chunk(cc):
            # one pass of the ET chunk computes V^T (rows 0:64) + K^T (64:128)
            ps = psb.tile([P, 512], f32, tag="pj", name=f"vk_ps_{cc}")
            for ko in range(KO):
                nc.tensor.matmul(
                    ps[:], wts_sb[:, ko, 0:128], ET[:, cc, ko, :],
                    start=(ko == 0), stop=(ko == KO - 1),
                )
            nc.vector.tensor_scalar_add(
                KT[H:P, cc * 512:(cc + 1) * 512], ps[H:P, :], bk_sb[H:P]
            )
            vt = vtp.tile([H, 512], bf16, tag="vt", name=f"vt_{cc}")
            nc.vector.tensor_scalar_add(vt[:], ps[:H, :], bv_sb[:])
            vts[cc] = vt

        def q_chunk(cc):
            # M=64 matmul targeting PSUM rows 64:128 so Q^T lands at base 64
            ps = psb.tile([P, 512], f32, tag="pj", name=f"q_ps_{cc}")
            for ko in range(KO):
                nc.tensor.matmul(
                    ps[H:P, :], wts_sb[:, ko, 128:192], ET[:, cc, ko, :],
                    start=(ko == 0), stop=(ko == KO - 1),
                )
            nc.vector.tensor_scalar_add(
                QT[H:P, cc * 512:(cc + 1) * 512], ps[H:P, :], bq_sb[H:P]
            )

        # --- attention ---
        outT0 = psb.tile([P, 512], f32, tag="os0", bufs=1)
        outT1 = psb.tile([P, 512], f32, tag="os1", bufs=1)
        pts = [None] * NT

        def sc_wide(kt):
            # kt 0..3 (slot0 tri-masked) and 8..11 (slot0 data-gated):
            # one [128,1024] score psum over 2 banks, a single exp
            ps = psb.tile(
                [P, 1024], f32, tag="sc", name=f"sc_{kt}", bufs=2
            )
            kblk = KT[H:P, kt * P:(kt + 1) * P]
            nc.tensor.matmul(
                ps[:, 0:512], kblk, QT[H:P, 0:512],
                start=True, stop=True, skip_group_check=True,
            )
            nc.tensor.matmul(
                ps[:, 512:1024], kblk, QT[H:P, 512:1024],
                start=True, stop=True, skip_group_check=True,
            )
            pt = ptp.tile([P, 1024], bf16, tag="pt", name=f"pt_{kt}")
            pts[kt] = (pt[:, 0:512], pt[:, 512:1024])
            nc.scalar.activation(pt[:], ps[:], AF.Exp)
            if kt < 4:
                nc.vector.tensor_tensor(
                    pt[:, 0:512], pt[:, 0:512], mask_ap(kt), ALU.mult
                )
            else:
                nc.vector.tensor_scalar_mul(
                    pt[:, 0:512], pt[:, 0:512], g8_sb[:]
                )

        def sc_pair(kta):
            # two slot1-only key tiles (kt 4..7 tri / 12..15 exp-bias
            # gated) share one psum + one exp
            ktb = kta + 1
            ps = psb.tile(
                [P, 1024], f32, tag="sc", name=f"sc_{kta}_{ktb}", bufs=2
            )
            for i, kt in enumerate((kta, ktb)):
                nc.tensor.matmul(
                    ps[:, 512 * i:512 * (i + 1)],
                    KT[H:P, kt * P:(kt + 1) * P], QT[H:P, 512:1024],
                    start=True, stop=True, skip_group_check=True,
                )
            pt = ptp.tile([P, 1024], bf16, tag="pt", name=f"pt_{kta}_{ktb}")
            pts[kta] = (None, pt[:, 0:512])
            pts[ktb] = (None, pt[:, 512:1024])
            if kta >= 12:
                nc.scalar.activation(pt[:], ps[:], AF.Exp, bias=g12_sb[:])
            else:
                nc.scalar.activation(pt[:], ps[:], AF.Exp)
                nc.vector.tensor_tensor(
                    pt[:, 0:512], pt[:, 0:512], mask_ap(kta - 4), ALU.mult
                )
                nc.vector.tensor_tensor(
                    pt[:, 512:1024], pt[:, 512:1024], mask_ap(ktb - 4), ALU.mult
                )

        def pv(kt):
            p0, p1 = pts[kt]
            if p0 is not None:
                nc.tensor.matmul(
                    outT0[:H + 1, :], Vp[:, kt, :], p0,
                    start=(kt == 0), stop=(kt == 11), skip_group_check=True,
                )
            nc.tensor.matmul(
                outT1[:H + 1, :], Vp[:, kt, :], p1,
                start=(kt == 0), stop=(kt == 15), skip_group_check=True,
            )

        # --- emission order = per-engine FIFO order; hand-pipelined so PE
        # never waits on ACT/DVE and ACT starts exping early ---
        vk_chunk0()
        q_chunk0()
        q_chunk(1)
        sc_wide(0)
        sc_wide(1)
        sc_wide(2)
        sc_wide(3)
        vk_chunk(1)
        sc_pair(4)
        vtranspose(vts[0], 0)
        sc_pair(6)
        vtranspose(vts[1], 1)
        pv(0)
        pv(1)
        pv(2)
        pv(3)
        vk_chunk(2)
        sc_wide(8)
        pv(4)
        sc_wide(9)
        pv(5)
        vtranspose(vts[2], 2)
        sc_wide(10)
        pv(6)
        sc_wide(11)
        pv(7)
        vk_chunk(3)
        sc_pair(12)
        pv(8)
        pv(9)
        vtranspose(vts[3], 3)
        sc_pair(14)
        pv(10)
        pv(11)
        nc.vector.tensor_copy(o_sb[:H + 1, 0:512], outT0[:H + 1, :])
        nc.sync.dma_start(out[:, 0:512], o_sb[:H + 1, 0:512])
        pv(12)
        pv(13)
        pv(14)
        pv(15)
        nc.vector.tensor_copy(o_sb[:H + 1, 512:1024], outT1[:H + 1, :])
        nc.sync.dma_start(out[:, 512:1024], o_sb[:H + 1, 512:1024])

    nc.finalize()
    return nc


_CACHED = None


def _get_program():
    global _CACHED
    if _CACHED is None:
        _CACHED = _build_program()
    return _CACHED


def _host_inputs(embeddings, Wq, bq, Wk, bk, Wv, bv):
    import ml_dtypes

    bf16 = ml_dtypes.bfloat16
    halves = _halves()
    # multiplicative tri masks, [k, j, c] layout: 1 where c >= k + j*128
    masks = np.zeros((P, 4, 512), np.float32)
    for j in range(4):
        for k in range(P):
            masks[k, j, k + j * P:] = 1.0
    ident = np.zeros((P, H), np.float32)
    ident[:H] = np.eye(H, dtype=np.float32)
    mi = np.ascontiguousarray(
        np.concatenate([masks.reshape(P, 4 * 512), ident], axis=1)
    ).astype(bf16)

    def wlay(w):
        return np.asarray(w, np.float32).reshape(KO, P, H).transpose(1, 0, 2)

    wq8l = wlay(Wq) / 8.0
    wkl = wlay(Wk)
    wvl = wlay(Wv)
    wts = np.ascontiguousarray(
        np.concatenate([wvl, wkl, wq8l], axis=2)
    ).astype(bf16)
    bqf = np.asarray(bq, np.float32) / 8.0
    bkf = np.asarray(bk, np.float32)
    bvf = np.asarray(bv, np.float32)
    z64 = np.zeros(H, np.float32)
    bq8P = np.concatenate([z64, bqf])
    bkP = np.concatenate([z64, bkf])
    bvP = np.concatenate([bvf, z64])

    in_maps = []
    perms = []
    for c in range(8):
        b, h = c // 2, c % 2
        own = halves[h]
        other = halves[1 - h]
        rows = np.concatenate(
            [np.arange(a, z) for a, z in own] + [np.arange(a, z) for a, z in other]
        )
        perms.append(rows)
        ep = embeddings[b][rows]                      # [S, D] f32, permuted
        etl = np.ascontiguousarray(
            ep.T.reshape(KO, P, 4, 512).transpose(2, 1, 0, 3)
        ).astype(bf16).reshape(4, P, KO * 512)        # [cc, p, ko*512]
        # chunk 0 again as four 128-col sub-chunks [j, p, ko, 128]
        et0l = np.ascontiguousarray(
            etl[0].reshape(P, KO, 4, P).transpose(2, 0, 1, 3)
        )
        g8v = np.full(P, 1.0 if h == 1 else 0.0, np.float32)
        g12v = np.full(P, NEG if h == 1 else 0.0, np.float32)
        bias4 = np.ascontiguousarray(
            np.stack([bq8P, bkP, g8v, g12v, bvP], axis=1)
        )
        in_maps.append({
            "et": etl, "et0": et0l, "wts": wts, "bias4": bias4, "mi": mi,
        })
    return in_maps, perms


def _run(embeddings, Wq, bq, Wk, bk, Wv, bv, trace=False):
    from concourse.bass_utils import run_bass_kernel_spmd

    nc = _get_program()
    in_maps, perms = _host_inputs(embeddings, Wq, bq, Wk, bk, Wv, bv)
    res = run_bass_kernel_spmd(
        nc, in_maps, core_ids=list(range(8)), trace=trace,
        trace_cores=list(range(8)) if trace else None,
    )
    full = np.empty((B, S, H), np.float32)
    for c in range(8):
        b = c // 2
        o = res.results[c]["out"]                     # [65, 1024] f32
        full[b, perms[c][:1024]] = (o[:H] / o[H:H + 1]).T
    return full, res


def kernel(embeddings, Wq, bq, Wk, bk, Wv, bv):
    full, _ = _run(
        np.asarray(embeddings, np.float32), Wq, bq, Wk, bk, Wv, bv, trace=False
    )
    return full
